# revision 1
# baseline (speedup 1.0000x reference)
"""Trainium2 Bass kernel for nn_AttentionSubModule: batched tiny attention.

Per item (131072 total): x row of 225 = 25 tokens x 9 dims, 4 token groups
each with own 9x9 Wq/Wk/Wv + bias; scores = qk^T/3 (+mask*-1e9), softmax,
out = attn@v + residual, LayerNorm over the 9-dim axis.

Mapping: pure data parallel over 8 cores (16384 items each), 128 items per
SBUF tile (items on partitions).

 - q/k/v projections on the PE: transpose x (PE transpose), multiply by
   block-diagonal per-token weight matrices (shared stationary), add biases
   during the PSUM->SBUF evacuation on the scalar engine (per-partition bias
   in the transposed layout), then PE-transpose back to item-rows.
 - scores and attn@v: vector-engine broadcast-AP multiplies + strided
   reduces (per-item 25x25x9 contractions don't map onto the PE).
 - exp on the scalar engine; softmax division folded away via LayerNorm
   scale invariance: LN(attn@v/Z + x) == LN(attn_unnorm@v + Z*x).
"""

import numpy as np
from contextlib import ExitStack

import concourse.bass as bass
import concourse.tile as tile
from concourse import mybir
from concourse.bass_utils import run_bass_kernel_spmd

KV = 9
NQ = 25
D = NQ * KV  # 225
GROUPS = [(0, 27, 3), (27, 117, 10), (117, 207, 10), (207, 225, 2)]
N_CORES = 8
P = 128
EPS = 1e-5
F32 = mybir.dt.float32
BF16 = mybir.dt.float16
SCORES_BF16 = True
ATTNV_BF16 = True

NA = 14 * KV   # chunk A: tokens 0..13 -> 126 rows
NB = 11 * KV   # chunk B: tokens 14..24 -> 99 rows

# pmat (per-partition consts) column layout:
#   [0:128)    identity 128x128
#   [128:254)  MqA  (126x126)    [254:380) MkA   [380:506) MvA
#   [506:605)  MqB  (99x99)      [605:704) MkB   [704:803) MvB
#   803 bqA | 804 bqB | 805 bkA | 806 bkB | 807 bvA | 808 bvB
PMAT_COLS = 128 + 3 * NA + 3 * NB + 6  # 809

# cst (broadcast consts): [mask 25 | gamma 9 | beta 9]
CST_LEN = NQ + KV + KV


def _bcast_ap(handle, n_part):
    ap = handle[:]
    return bass.AP(tensor=ap.tensor, offset=ap.offset, ap=[[0, n_part]] + list(ap.ap))


def build_program(b_core, probe=3):
    assert b_core % P == 0
    ntiles = b_core // P
    nc = bass.Bass("TRN2", target_bir_lowering=False)

    x_d = nc.dram_tensor("x", [b_core, D], F32, kind="ExternalInput")
    cst_d = nc.dram_tensor("cst", [CST_LEN], F32, kind="ExternalInput")
    pmat_d = nc.dram_tensor("pmat", [P, PMAT_COLS], F32, kind="ExternalInput")
    out_d = nc.dram_tensor("out", [b_core, D], F32, kind="ExternalOutput")

    with tile.TileContext(nc) as tc, ExitStack() as ctx:
        consts = ctx.enter_context(tc.tile_pool(name="consts", bufs=1))
        xin = ctx.enter_context(tc.tile_pool(name="xin", bufs=4))
        tlay = ctx.enter_context(tc.tile_pool(name="tlay", bufs=2))
        proj = ctx.enter_context(tc.tile_pool(name="proj", bufs=3))
        big = ctx.enter_context(tc.tile_pool(name="big", bufs=3))
        sm = ctx.enter_context(tc.tile_pool(name="sm", bufs=3))
        outp = ctx.enter_context(tc.tile_pool(name="outp", bufs=3))
        psum = ctx.enter_context(tc.tile_pool(name="psum", bufs=8, space="PSUM"))

        # ---- constants ----
        cst_t = consts.tile([P, CST_LEN], F32)
        nc.gpsimd.dma_start(out=cst_t, in_=_bcast_ap(cst_d, P))
        m_t = cst_t[:, 0:NQ]
        g_t = cst_t[:, NQ : NQ + KV]
        b_t = cst_t[:, NQ + KV : NQ + 2 * KV]

        pm_t = consts.tile([P, PMAT_COLS], F32)
        nc.sync.dma_start(out=pm_t, in_=pmat_d[:, :])
        ident = pm_t[:, 0:128]
        mA = {}
        mB = {}
        o = 128
        for nm in ("q", "k", "v"):
            mA[nm] = pm_t[0:NA, o : o + NA]; o += NA
        for nm in ("q", "k", "v"):
            mB[nm] = pm_t[0:NB, o : o + NB]; o += NB
        biasA = {}
        biasB = {}
        for nm in ("q", "k", "v"):
            biasA[nm] = pm_t[0:NA, o : o + 1]; o += 1
            biasB[nm] = pm_t[0:NB, o : o + 1]; o += 1
        assert o == PMAT_COLS

        # expm[p, j] = exp(-1e9 * mask[j]); multiplied into exp(scores)
        expm_t = consts.tile([P, NQ], BF16 if ATTNV_BF16 else F32)
        nc.scalar.activation(
            expm_t[:], m_t, mybir.ActivationFunctionType.Exp, bias=0.0, scale=-1e9
        )
        eps_t = consts.tile([P, 1], F32)
        nc.vector.memset(eps_t[:], EPS)
        # constant exp-shift: exp(s - 8) keeps fp16 attention weights and
        # partial sums in range (max score ~16.4); softmax is shift-invariant
        # and the LN scale-invariance absorbs the global factor exactly.
        shift_t = consts.tile([P, 1], F32)
        nc.vector.memset(shift_t[:], -8.0)
        # wait-absorbers: sync DVE/ACT on the const DMAs via tiny copies so the
        # wide TensorTensor encodings never need more than one sync-wait.
        absorb_t = consts.tile([P, 4], F32)
        nc.vector.tensor_copy(absorb_t[:], cst_t[:, 0:4])
        absorb2_t = consts.tile([P, 4], F32)
        nc.vector.tensor_copy(absorb2_t[:], pm_t[:, 0:4])

        inv_sqrt_kv = float(1.0 / np.sqrt(KV))
        AF = mybir.ActivationFunctionType

        for t in range(ntiles):
            xt = xin.tile([P, D], F32)
            nc.sync.dma_start(out=xt, in_=x_d[t * P : (t + 1) * P, :])
            xv = xt[:].rearrange("p (i d) -> p i d", i=NQ)

            if probe == 0:
                o_t0 = outp.tile([P, NQ, KV], F32, tag="o")
                nc.vector.tensor_copy(o_t0[:].rearrange("p a b -> p (a b)"), xt[:])
                nc.sync.dma_start(
                    out=out_d[t * P : (t + 1) * P, :],
                    in_=o_t0[:].rearrange("p a b -> p (a b)"),
                )
                continue
            # ---- projections on PE (transposed layout) ----
            xT1p = psum.tile([P, P], F32, tag="ps")
            xT2p = psum.tile([P, P], F32, tag="ps")
            nc.tensor.transpose(xT1p[0:NA, :], xt[:, 0:NA], ident)
            nc.tensor.transpose(xT2p[0:NB, :], xt[:, NA:D], ident)
            xT1 = tlay.tile([P, P], F32, tag="xT1")
            xT2 = tlay.tile([P, P], F32, tag="xT2")
            nc.scalar.copy(xT1[0:NA, :], xT1p[0:NA, :])
            nc.scalar.copy(xT2[0:NB, :], xT2p[0:NB, :])

            nat = {}
            for nm in ("q", "k", "v"):
                pA = psum.tile([P, P], F32, tag="ps")
                pB = psum.tile([P, P], F32, tag="ps")
                nc.tensor.matmul(pA[0:NA, :], mA[nm], xT1[0:NA, :], start=True, stop=True)
                nc.tensor.matmul(pB[0:NB, :], mB[nm], xT2[0:NB, :], start=True, stop=True)
                # evacuate with bias add (per-partition bias in T layout)
                sT1 = tlay.tile([P, P], F32, tag=f"s{nm}1")
                sT2 = tlay.tile([P, P], F32, tag=f"s{nm}2")
                nc.scalar.activation(sT1[0:NA, :], pA[0:NA, :], AF.Identity,
                                     bias=biasA[nm], scale=1.0)
                nc.scalar.activation(sT2[0:NB, :], pB[0:NB, :], AF.Identity,
                                     bias=biasB[nm], scale=1.0)
                # transpose back to item-rows
                nA = psum.tile([P, P], F32, tag="ps")
                nB_ = psum.tile([P, P], F32, tag="ps")
                nc.tensor.transpose(nA[:, 0:NA], sT1[0:NA, :], ident[0:NA, 0:NA])
                nc.tensor.transpose(nB_[:, 0:NB], sT2[0:NB, :], ident[0:NB, 0:NB])
                want_bf = (SCORES_BF16 and nm in ("q", "k")) or (ATTNV_BF16 and nm == "v")
                dst = proj.tile([P, NQ, KV], BF16 if want_bf else F32, tag=nm)
                flat = dst[:].rearrange("p a b -> p (a b)")
                nc.scalar.copy(flat[:, 0:NA], nA[:, 0:NA])
                nc.scalar.copy(flat[:, NA:D], nB_[:, 0:NB])
                nat[nm] = dst

            q_t, k_t, v_t = nat["q"], nat["k"], nat["v"]

            if probe == 1:
                o_t1 = outp.tile([P, NQ, KV], F32, tag="o")
                nc.vector.tensor_add(o_t1[:], q_t[:], k_t[:])
                nc.sync.dma_start(
                    out=out_d[t * P : (t + 1) * P, :],
                    in_=o_t1[:].rearrange("p a b -> p (a b)"),
                )
                continue

            # ---- scores + exp (no max-subtraction: |scores| <~ 15) ----
            pr2 = big.tile([P, NQ, NQ, KV], BF16 if SCORES_BF16 else F32, tag="bigprod")
            nc.vector.tensor_mul(
                pr2[:],
                q_t[:].unsqueeze(2).broadcast_to((P, NQ, NQ, KV)),
                k_t[:].unsqueeze(1).broadcast_to((P, NQ, NQ, KV)),
            )
            sc = sm.tile([P, NQ, NQ], F32, tag="sc")
            if SCORES_BF16:
                # strided TT-tree reduce over d (faster than 1x tensor_reduce)
                PD = BF16
                t1 = sm.tile([P, NQ * NQ, 4], PD, tag="sct1")
                p4 = pr2[:].rearrange("p a b d -> p (a b) d")
                nc.vector.tensor_add(t1[:], p4[:, :, 0:4], p4[:, :, 4:8])
                t2 = sm.tile([P, NQ * NQ, 2], PD, tag="sct2")
                nc.vector.tensor_add(t2[:], t1[:, :, 0:2], t1[:, :, 2:4])
                t3 = sm.tile([P, NQ * NQ, 1], PD, tag="sct3")
                nc.vector.tensor_add(t3[:], t2[:, :, 0:1], t2[:, :, 1:2])
                nc.vector.tensor_add(
                    sc[:].rearrange("p a b -> p (a b)").unsqueeze(2),
                    t3[:], p4[:, :, 8:9],
                )
            else:
                nc.vector.tensor_reduce(
                    sc[:], pr2[:], axis=mybir.AxisListType.X, op=mybir.AluOpType.add
                )
            ex = sm.tile([P, NQ, NQ], BF16 if ATTNV_BF16 else F32, tag="ex")
            nc.scalar.activation(
                ex[:].rearrange("p a b -> p (a b)"),
                sc[:].rearrange("p a b -> p (a b)"),
                AF.Exp, bias=shift_t[:], scale=inv_sqrt_kv,
            )
            # apply mask weights: e'[p,i,j] = e[p,i,j] * expm[p,j]
            nc.vector.tensor_mul(
                ex[:], ex[:], expm_t[:].unsqueeze(1).broadcast_to((P, NQ, NQ))
            )
            # Z[p, i] = sum_j e'
            z_t = sm.tile([P, NQ], F32, tag="z")
            nc.vector.tensor_reduce(
                z_t[:], ex[:], axis=mybir.AxisListType.X, op=mybir.AluOpType.add
            )

            if probe == 2:
                o_t2 = outp.tile([P, NQ, KV], F32, tag="o")
                nc.vector.tensor_mul(
                    o_t2[:], v_t[:], z_t[:].unsqueeze(2).broadcast_to((P, NQ, KV))
                )
                nc.sync.dma_start(
                    out=out_d[t * P : (t + 1) * P, :],
                    in_=o_t2[:].rearrange("p a b -> p (a b)"),
                )
                continue
            # ---- unnormalized attn @ v: un[p,i,e] = sum_j e'[p,i,j] v[p,j,e] ----
            # vE: v reordered (e-major) so the products-TT innermost stride is 1
            vE = proj.tile([P, KV, NQ], BF16 if ATTNV_BF16 else F32, tag="ve")
            nc.scalar.copy(vE[:], v_t[:].transpose([0, 2, 1]))
            pr3 = big.tile([P, NQ, KV, NQ], BF16 if ATTNV_BF16 else F32, tag="bigprod")
            nc.vector.tensor_mul(
                pr3[:],
                ex[:].unsqueeze(2).broadcast_to((P, NQ, KV, NQ)),
                vE[:].unsqueeze(1).broadcast_to((P, NQ, KV, NQ)),
            )
            u_t = outp.tile([P, NQ, KV], F32, tag="u")
            if ATTNV_BF16:
                PD = BF16
                q3 = pr3[:].rearrange("p a b d -> p (a b) d")
                r1 = sm.tile([P, NQ * KV, 12], PD, tag="avr1")
                nc.vector.tensor_add(r1[:], q3[:, :, 0:12], q3[:, :, 12:24])
                r2 = sm.tile([P, NQ * KV, 6], PD, tag="avr2")
                nc.vector.tensor_add(r2[:], r1[:, :, 0:6], r1[:, :, 6:12])
                r3 = sm.tile([P, NQ * KV, 3], PD, tag="avr3")
                nc.vector.tensor_add(r3[:], r2[:, :, 0:3], r2[:, :, 3:6])
                r4 = sm.tile([P, NQ * KV, 1], PD, tag="avr4")
                nc.vector.tensor_add(r4[:], r3[:, :, 0:1], r3[:, :, 1:2])
                r5 = sm.tile([P, NQ * KV, 1], PD, tag="avr5")
                nc.vector.tensor_add(r5[:], r4[:], r3[:, :, 2:3])
                nc.vector.tensor_add(
                    u_t[:].rearrange("p a b -> p (a b)").unsqueeze(2),
                    r5[:], q3[:, :, 24:25],
                )
            else:
                nc.vector.tensor_reduce(
                    u_t[:], pr3[:], axis=mybir.AxisListType.X, op=mybir.AluOpType.add
                )
            # u += Z * x   (residual, scaled by Z; LN is scale-invariant)
            zx = outp.tile([P, NQ, KV], F32, tag="zx")
            nc.vector.tensor_mul(
                zx[:], xv, z_t[:].unsqueeze(2).broadcast_to((P, NQ, KV))
            )
            nc.vector.tensor_add(u_t[:], u_t[:], zx[:])

            # ---- LayerNorm over e (9) ----
            s_t = sm.tile([P, NQ], F32, tag="lnsum")
            nc.vector.tensor_reduce(
                s_t[:], u_t[:], axis=mybir.AxisListType.X, op=mybir.AluOpType.add
            )
            mu = sm.tile([P, NQ], F32, tag="mu")
            nc.scalar.mul(mu[:], s_t[:], 1.0 / KV)
            cen = outp.tile([P, NQ, KV], F32, tag="cen")
            nc.vector.tensor_sub(
                cen[:], u_t[:], mu[:].unsqueeze(2).broadcast_to((P, NQ, KV))
            )
            sq = outp.tile([P, NQ, KV], F32, tag="sq")
            nc.scalar.square(
                sq[:].rearrange("p a b -> p (a b)"), cen[:].rearrange("p a b -> p (a b)")
            )
            vs = sm.tile([P, NQ], F32, tag="vs")
            nc.vector.tensor_reduce(
                vs[:], sq[:], axis=mybir.AxisListType.X, op=mybir.AluOpType.add
            )
            # scale-correct eps: u = Z*out_ref, so var_ref = var_u / Z^2;
            # rstd_ref/Z = 1/sqrt(var_u + Z^2*eps)
            zsq = sm.tile([P, NQ], F32, tag="zsq")
            nc.scalar.square(zsq[:], z_t[:])
            vs2 = sm.tile([P, NQ], F32, tag="vs2")
            nc.vector.scalar_tensor_tensor(
                vs2[:], zsq[:], float(KV * EPS), vs[:],
                op0=mybir.AluOpType.mult, op1=mybir.AluOpType.add,
            )
            sd = sm.tile([P, NQ], F32, tag="sd")
            nc.scalar.activation(
                sd[:], vs2[:], AF.Sqrt, bias=0.0, scale=1.0 / KV
            )
            rstd = sm.tile([P, NQ], F32, tag="rstd")
            nc.vector.reciprocal(rstd[:], sd[:])

            o_t = outp.tile([P, NQ, KV], F32, tag="o")
            nc.vector.tensor_mul(
                o_t[:], cen[:], rstd[:].unsqueeze(2).broadcast_to((P, NQ, KV))
            )
            nc.vector.tensor_mul(
                o_t[:], o_t[:], g_t.unsqueeze(1).broadcast_to((P, NQ, KV))
            )
            nc.vector.tensor_add(
                o_t[:], o_t[:], b_t.unsqueeze(1).broadcast_to((P, NQ, KV))
            )
            nc.sync.dma_start(
                out=out_d[t * P : (t + 1) * P, :],
                in_=o_t[:].rearrange("p a b -> p (a b)"),
            )

    _split_multi_waits(nc)
    return nc


def _split_multi_waits(nc):
    """Walrus allows only one sync-wait slot on most instruction encodings.
    Hoist excess waits into NoOps inserted just before the offender (same
    engine, same block => same ordering semantics)."""
    for f in nc.m.functions:
        for b in f.blocks:
            i = 0
            while i < len(b.instructions):
                inst = b.instructions[i]
                si = getattr(inst, "sync_info", None)
                if si is not None and si.on_wait and len(si.on_wait) > 1:
                    extra = si.on_wait[:-1]
                    si.on_wait = si.on_wait[-1:]
                    for w in extra:
                        nop = mybir.InstNoOp(
                            name=nc.get_next_instruction_name(),
                            engine=inst.engine,
                            ins=[],
                            outs=[],
                            sync_info=mybir.SyncInfo(on_wait=[w], on_update=[]),
                        )
                        nc.register_instruction(nop)
                        b.instructions.insert(i, nop)
                        i += 1
                i += 1
    return nc


_NC_CACHE = {}


def _get_program(b_core):
    if b_core not in _NC_CACHE:
        _NC_CACHE[b_core] = build_program(b_core)
    return _NC_CACHE[b_core]


def _host_consts(Wq, bq, Wk, bk, Wv, bv):
    """Build pmat [128, PMAT_COLS]: identity, block-diag projection mats
    (transposed-layout), bias columns."""
    gidx = np.empty(NQ, dtype=np.int64)
    for g, (s, e, n) in enumerate(GROUPS):
        gidx[s // KV : e // KV] = g

    def mk_blockdiag(W, tok_lo, tok_hi):
        n = (tok_hi - tok_lo) * KV
        M = np.zeros((n, n), dtype=np.float32)
        for i in range(tok_lo, tok_hi):
            blk = W[gidx[i]]  # [e, d]
            r = (i - tok_lo) * KV
            # lhsT[(n,d'), (i,e)] = W[g(i)][e, d']  -> block at [r:r+9, r:r+9] = W.T
            M[r : r + KV, r : r + KV] = blk.T
        return M

    def mk_bias(b_, tok_lo, tok_hi):
        return np.concatenate([b_[gidx[i]] for i in range(tok_lo, tok_hi)]).astype(
            np.float32
        )

    pmat = np.zeros((P, PMAT_COLS), dtype=np.float32)
    pmat[:, 0:128] = np.eye(P, dtype=np.float32)
    o = 128
    for W in (Wq, Wk, Wv):
        pmat[0:NA, o : o + NA] = mk_blockdiag(np.asarray(W, np.float32), 0, 14)
        o += NA
    for W in (Wq, Wk, Wv):
        pmat[0:NB, o : o + NB] = mk_blockdiag(np.asarray(W, np.float32), 14, 25)
        o += NB
    for b_ in (bq, bk, bv):
        pmat[0:NA, o] = mk_bias(np.asarray(b_, np.float32), 0, 14); o += 1
        pmat[0:NB, o] = mk_bias(np.asarray(b_, np.float32), 14, 25); o += 1
    assert o == PMAT_COLS
    return pmat


def kernel(x, mask, Wq, bq, Wk, bk, Wv, bv, gamma, beta):
    x = np.ascontiguousarray(np.asarray(x, dtype=np.float32))
    B = x.shape[0]
    b_core = B // N_CORES
    pmat = _host_consts(Wq, bq, Wk, bk, Wv, bv)
    cst = np.concatenate([
        np.asarray(mask, dtype=np.float32).reshape(-1),
        np.asarray(gamma, dtype=np.float32).reshape(-1),
        np.asarray(beta, dtype=np.float32).reshape(-1),
    ]).astype(np.float32)
    assert cst.shape[0] == CST_LEN

    nc = _get_program(b_core)
    shards = x.reshape(N_CORES, b_core, D)
    in_maps = []
    for c in range(N_CORES):
        in_maps.append({
            "x": np.ascontiguousarray(shards[c]),
            "cst": cst,
            "pmat": pmat,
        })
    res = run_bass_kernel_spmd(nc, in_maps, core_ids=list(range(N_CORES)))
    outs = [res.results[c]["out"] for c in range(N_CORES)]
    full = np.concatenate(outs, axis=0).reshape(B, NQ, KV)
    return full.astype(np.float32)



# revision 14
# speedup vs baseline: 1.7503x; 1.7503x over previous
"""Trainium2 Bass kernel for nn_AttentionSubModule: batched tiny attention.

Per item (131072 total): x row of 225 = 25 tokens x 9 dims, 4 token groups
each with own 9x9 Wq/Wk/Wv + bias; scores = qk^T/3 (+mask*-1e9), softmax,
out = attn@v + residual, LayerNorm over the 9-dim axis.

Mapping: pure data parallel over 8 cores (16384 items each), 128 items per
SBUF tile (items on partitions), 3-stage software pipeline per tile:

  proj(t+1)  PE transposes + block-diag projection matmuls, ACT evacs
  main(t)    scores products + d-tree, exp, attn@v products + j-tree.
             The big products are SPLIT by query token between the vector
             (DVE) and gpsimd (Pool) engines to balance occupancy.
  back(t-1)  scaled residual, LayerNorm, output DMA (Pool + DVE + ACT).

Tricks:
 - softmax division folded away via LayerNorm scale invariance:
   LN(attn@v/Z + x) == LN(attn_unnorm@v + Z*x).
 - mask weights exp(-1e9*mask) folded host-side into the v projection
   blocks; Z comes out of the attn@v tree via an extra expm row in vE.
 - mean-subtraction sign absorbed into a host-negated gamma so the
   centering step is a single scalar_tensor_tensor.
"""

import numpy as np
from contextlib import ExitStack

import concourse.bass as bass
import concourse.tile as tile
from concourse import mybir
from concourse.bass_utils import run_bass_kernel_spmd

KV = 9
NQ = 25
D = NQ * KV  # 225
GROUPS = [(0, 27, 3), (27, 117, 10), (117, 207, 10), (207, 225, 2)]
N_CORES = 8
P = 128
EPS = 1e-5
F32 = mybir.dt.float32
F16 = mybir.dt.float16

NA = 14 * KV   # chunk A: tokens 0..13 -> 126 rows
NB = 11 * KV   # chunk B: tokens 14..24 -> 99 rows

# DVE/Pool split points: query tokens [0,CUT) of the scores / attn@v
# products go to DVE, [CUT,25) to Pool (gpsimd).
CUT2 = 20
CUT3 = 20

# pmat (per-partition consts) column layout:
#   [0:128)    identity 128x128
#   [128:254)  MqA  (126x126)    [254:380) MkA   [380:506) MvA
#   [506:605)  MqB  (99x99)      [605:704) MkB   [704:803) MvB
#   803 bqA | 804 bqB | 805 bkA | 806 bkB | 807 bvA | 808 bvB
PMAT_COLS = 128 + 3 * NA + 3 * NB + 6  # 809

# cst (broadcast consts): [expm 25 | -gamma 9 | beta 9]
CST_LEN = NQ + KV + KV


def _bcast_ap(handle, n_part):
    ap = handle[:]
    return bass.AP(tensor=ap.tensor, offset=ap.offset, ap=[[0, n_part]] + list(ap.ap))


def build_program(b_core):
    assert b_core % P == 0
    ntiles = b_core // P
    nc = bass.Bass("TRN2", target_bir_lowering=False)

    x_d = nc.dram_tensor("x", [b_core, D], F32, kind="ExternalInput")
    cst_d = nc.dram_tensor("cst", [CST_LEN], F32, kind="ExternalInput")
    pmat_d = nc.dram_tensor("pmat", [P, PMAT_COLS], F32, kind="ExternalInput")
    out_d = nc.dram_tensor("out", [b_core, D], F32, kind="ExternalOutput")

    with tile.TileContext(nc) as tc, ExitStack() as ctx:
        consts = ctx.enter_context(tc.tile_pool(name="consts", bufs=1))
        xin = ctx.enter_context(tc.tile_pool(name="xin", bufs=6))
        tlay = ctx.enter_context(tc.tile_pool(name="tlay", bufs=2))
        proj = ctx.enter_context(tc.tile_pool(name="proj", bufs=4))
        big = ctx.enter_context(tc.tile_pool(name="big", bufs=3))
        sm = ctx.enter_context(tc.tile_pool(name="sm", bufs=3))
        outp = ctx.enter_context(tc.tile_pool(name="outp", bufs=3))
        psum = ctx.enter_context(tc.tile_pool(name="psum", bufs=2, space="PSUM"))
        pacc = ctx.enter_context(tc.tile_pool(name="pacc", bufs=2, space="PSUM"))

        # ---- constants ----
        cst_t = consts.tile([P, CST_LEN], F32)
        nc.gpsimd.dma_start(out=cst_t, in_=_bcast_ap(cst_d, P))
        m_t = cst_t[:, 0:NQ]          # expm = exp(-1e9*mask), host-computed
        g_t = cst_t[:, NQ : NQ + KV]  # -gamma (sign absorbs mu-u centering)
        b_t = cst_t[:, NQ + KV : NQ + 2 * KV]

        pm_t = consts.tile([P, PMAT_COLS], F32)
        nc.sync.dma_start(out=pm_t, in_=pmat_d[:, :])
        ident = pm_t[:, 0:128]
        mA = {}
        mB = {}
        o = 128
        for nm in ("q", "k", "v"):
            mA[nm] = pm_t[0:NA, o : o + NA]; o += NA
        for nm in ("q", "k", "v"):
            mB[nm] = pm_t[0:NB, o : o + NB]; o += NB
        biasA = {}
        biasB = {}
        for nm in ("q", "k", "v"):
            biasA[nm] = pm_t[0:NA, o : o + 1]; o += 1
            biasB[nm] = pm_t[0:NB, o : o + 1]; o += 1
        assert o == PMAT_COLS

        # fp16 identity: stationary for PE accumulate-copy reduction matmuls
        identF16 = consts.tile([P, P], F16)
        nc.vector.tensor_copy(identF16[:], ident)

        # constant exp-shift: exp(s - 8) keeps fp16 attention weights and
        # partial sums in range (max score ~16.4); softmax is shift-invariant
        # and the LN scale-invariance absorbs the global factor exactly.
        shift_t = consts.tile([P, 1], F32)
        nc.vector.memset(shift_t[:], -8.0)
        # wait-absorbers: sync engines on the const DMAs via tiny copies so
        # wide TensorTensor encodings never need more than one sync-wait.
        absorb_t = consts.tile([P, 4], F32)
        nc.vector.tensor_copy(absorb_t[:], cst_t[:, 0:4])
        absorb2_t = consts.tile([P, 4], F32)
        nc.vector.tensor_copy(absorb2_t[:], pm_t[:, 0:4])
        absorb3_t = consts.tile([P, 4], F32)
        nc.gpsimd.tensor_copy(absorb3_t[:], cst_t[:, 0:4])

        inv_sqrt_kv = float(1.0 / np.sqrt(KV))
        AF = mybir.ActivationFunctionType
        st = {}  # per-tile live handles

        def proj_stage(t):
            xt = xin.tile([P, D], F32, tag="x")
            nc.sync.dma_start(out=xt, in_=x_d[t * P : (t + 1) * P, :])

            xT1p = psum.tile([P, P], F32, tag="ps")
            xT2p = psum.tile([P, P], F32, tag="ps")
            nc.tensor.transpose(xT1p[0:NA, :], xt[:, 0:NA], ident)
            nc.tensor.transpose(xT2p[0:NB, :], xt[:, NA:D], ident)
            xT1 = tlay.tile([P, P], F32, tag="xT1")
            xT2 = tlay.tile([P, P], F32, tag="xT2")
            nc.scalar.copy(xT1[0:NA, :], xT1p[0:NA, :])
            nc.scalar.copy(xT2[0:NB, :], xT2p[0:NB, :])

            cur = {"x": xt}
            for nm in ("q", "k", "v"):
                pA = psum.tile([P, P], F32, tag="ps")
                pB = psum.tile([P, P], F32, tag="ps")
                nc.tensor.matmul(pA[0:NA, :], mA[nm], xT1[0:NA, :], start=True, stop=True)
                nc.tensor.matmul(pB[0:NB, :], mB[nm], xT2[0:NB, :], start=True, stop=True)
                sT1 = tlay.tile([P, P], F32, tag=f"s{nm}1")
                sT2 = tlay.tile([P, P], F32, tag=f"s{nm}2")
                nc.scalar.activation(sT1[0:NA, :], pA[0:NA, :], AF.Identity,
                                     bias=biasA[nm], scale=1.0)
                nc.scalar.activation(sT2[0:NB, :], pB[0:NB, :], AF.Identity,
                                     bias=biasB[nm], scale=1.0)
                nA = psum.tile([P, P], F32, tag="ps")
                nB_ = psum.tile([P, P], F32, tag="ps")
                nc.tensor.transpose(nA[:, 0:NA], sT1[0:NA, :], ident[0:NA, 0:NA])
                nc.tensor.transpose(nB_[:, 0:NB], sT2[0:NB, :], ident[0:NB, 0:NB])
                if nm in ("q", "k"):
                    dst = proj.tile([P, NQ, KV], F16, tag=nm)
                    flat = dst[:].rearrange("p a b -> p (a b)")
                    nc.scalar.copy(flat[:, 0:NA], nA[:, 0:NA])
                    nc.scalar.copy(flat[:, NA:D], nB_[:, 0:NB])
                    cur[nm] = dst
                else:
                    # v straight to e-major layout [P, 10, 25]:
                    # rows 0..8 = premasked v, row 9 = expm weights (-> Z)
                    vE = proj.tile([P, KV + 1, NQ], F16, tag="ve")
                    nc.scalar.copy(
                        vE[:, 0:KV, 0:14].transpose([0, 2, 1]),
                        nA[:, 0:NA].rearrange("p (j e) -> p j e", j=14),
                    )
                    nc.scalar.copy(
                        vE[:, 0:KV, 14:NQ].transpose([0, 2, 1]),
                        nB_[:, 0:NB].rearrange("p (j e) -> p j e", j=11),
                    )
                    nc.scalar.copy(vE[:, KV : KV + 1, :], m_t.unsqueeze(1))
                    cur["ve"] = vE
            st[t] = cur

        NSA = 13 * NQ              # scores columns split at the PSUM bank
        NSB = NQ * NQ - NSA
        NR = NQ * (KV + 1)         # 250 attn@v reduce groups

        def scores_stage(t):
            # pr2 products + pair-add, split DVE/Pool by query token (each
            # engine's chain is local); PE accumulates 5 slices -> PSUM f32.
            cur = st[t]
            q_t, k_t = cur["q"], cur["k"]
            pr2 = big.tile([P, NQ, NQ, KV], F16, tag="pr2")
            qb = q_t[:].unsqueeze(2).broadcast_to((P, NQ, NQ, KV))
            kb = k_t[:].unsqueeze(1).broadcast_to((P, NQ, NQ, KV))
            t1 = sm.tile([P, NQ, NQ, 4], F16, tag="sct1")
            nc.vector.tensor_mul(pr2[:, 0:CUT2], qb[:, 0:CUT2], kb[:, 0:CUT2])
            nc.vector.tensor_add(t1[:, 0:CUT2], pr2[:, 0:CUT2, :, 0:4],
                                 pr2[:, 0:CUT2, :, 4:8])
            nc.gpsimd.tensor_mul(pr2[:, CUT2:NQ], qb[:, CUT2:NQ], kb[:, CUT2:NQ])
            nc.gpsimd.tensor_add(t1[:, CUT2:NQ], pr2[:, CUT2:NQ, :, 0:4],
                                 pr2[:, CUT2:NQ, :, 4:8])
            t1f = t1[:].rearrange("p a b c -> p (a b) c")
            p4 = pr2[:].rearrange("p a b d -> p (a b) d")
            scA = pacc.tile([P, NSA], F32, tag="scA")
            scB = pacc.tile([P, NSB], F32, tag="scB")
            for lo, n, sct in ((0, NSA, scA), (NSA, NSB, scB)):
                srcs = [t1f[:, lo : lo + n, c : c + 1] for c in range(4)]
                srcs.append(p4[:, lo : lo + n, 8:9])
                for ci, s in enumerate(srcs):
                    nc.tensor.matmul(sct[:], identF16[:], s,
                                     start=(ci == 0), stop=(ci == len(srcs) - 1))
            cur["scA"], cur["scB"] = scA, scB

        def exp_stage(t):
            # exp from PSUM scores (inputs one period old -> ACT queue head)
            cur = st[t]
            scA, scB = cur.pop("scA"), cur.pop("scB")
            ex = sm.tile([P, NQ, NQ], F16, tag="ex")
            exf = ex[:].rearrange("p a b -> p (a b)")
            nc.scalar.activation(exf[:, 0:NSA], scA[:],
                                 AF.Exp, bias=shift_t[:], scale=inv_sqrt_kv)
            nc.scalar.activation(exf[:, NSA : NQ * NQ], scB[:],
                                 AF.Exp, bias=shift_t[:], scale=inv_sqrt_kv)
            cur["ex"] = ex

        def attnv_stage(t):
            # pr3 products + pair-add (DVE/Pool local chains),
            # PE accumulates 13 slices -> u10 PSUM (evac next period).
            # u10[p,i,e] = sum_j ex[p,i,j] vE[p,e,j]; e==9 row gives Z.
            cur = st[t]
            vE = cur["ve"]
            ex = cur.pop("ex")
            pr3 = big.tile([P, NQ, KV + 1, NQ], F16, tag="pr3")
            exb = ex[:].unsqueeze(2).broadcast_to((P, NQ, KV + 1, NQ))
            veb = vE[:].unsqueeze(1).broadcast_to((P, NQ, KV + 1, NQ))
            r1 = sm.tile([P, NQ, KV + 1, 12], F16, tag="avr1")
            nc.vector.tensor_mul(pr3[:, 0:CUT3], exb[:, 0:CUT3], veb[:, 0:CUT3])
            nc.vector.tensor_add(r1[:, 0:CUT3], pr3[:, 0:CUT3, :, 0:12],
                                 pr3[:, 0:CUT3, :, 12:24])
            nc.gpsimd.tensor_mul(pr3[:, CUT3:NQ], exb[:, CUT3:NQ], veb[:, CUT3:NQ])
            nc.gpsimd.tensor_add(r1[:, CUT3:NQ], pr3[:, CUT3:NQ, :, 0:12],
                                 pr3[:, CUT3:NQ, :, 12:24])
            r1f = r1[:].rearrange("p a b c -> p (a b) c")
            q3 = pr3[:].rearrange("p a b d -> p (a b) d")
            u10p = pacc.tile([P, NR], F32, tag="u10p")
            srcs = [r1f[:, :, c : c + 1] for c in range(12)]
            srcs.append(q3[:, :, 24:25])
            for ci, s in enumerate(srcs):
                nc.tensor.matmul(u10p[:], identF16[:], s,
                                 start=(ci == 0), stop=(ci == len(srcs) - 1))
            cur["u10p"] = u10p

        def ln1_stage(t):
            # u10 PSUM evac (inputs one period old), then residual +
            # LayerNorm stats (DVE-local chain + ACT squares)
            cur = st[t]
            u10p = cur.pop("u10p")
            u10 = outp.tile([P, NQ, KV + 1], F32, tag="u10")
            nc.scalar.copy(u10[:].rearrange("p a b -> p (a b)"), u10p[:])
            xv = cur["x"][:].rearrange("p (i d) -> p i d", i=NQ)
            uv = u10[:, :, 0:KV]           # [P, 25, 9] strided (groups of 10)
            zv = u10[:, :, KV : KV + 1]    # [P, 25, 1] Z

            # u += Z * x   (residual, scaled by Z; LN is scale-invariant)
            zx = outp.tile([P, NQ, KV], F32, tag="zx")
            nc.vector.tensor_mul(zx[:], xv, zv.broadcast_to((P, NQ, KV)))
            nc.vector.tensor_add(uv, uv, zx[:])
            zsq = sm.tile([P, NQ], F32, tag="zsq")
            nc.scalar.square(zsq[:], zv.rearrange("p a b -> p (a b)"))

            s_t = sm.tile([P, NQ], F32, tag="lnsum")
            nc.vector.tensor_reduce(
                s_t[:], uv, axis=mybir.AxisListType.X, op=mybir.AluOpType.add
            )
            # cen' = mu - u  (negated centering; sign folded into -gamma)
            cen = outp.tile([P, NQ, KV], F32, tag="cen")
            nc.vector.scalar_tensor_tensor(
                cen[:], s_t[:].unsqueeze(2).broadcast_to((P, NQ, KV)),
                float(1.0 / KV), uv,
                op0=mybir.AluOpType.mult, op1=mybir.AluOpType.subtract,
            )
            sq = outp.tile([P, NQ, KV], F32, tag="sq")
            nc.scalar.square(
                sq[:].rearrange("p a b -> p (a b)"), cen[:].rearrange("p a b -> p (a b)")
            )
            vs = sm.tile([P, NQ], F32, tag="vs")
            nc.vector.tensor_reduce(
                vs[:], sq[:], axis=mybir.AxisListType.X, op=mybir.AluOpType.add
            )
            # scale-correct eps: u = Z*out_ref, so var_ref = var_u / Z^2;
            # rstd_ref/Z = 1/sqrt(var_u + Z^2*eps)
            vs2 = sm.tile([P, NQ], F32, tag="vs2")
            nc.vector.scalar_tensor_tensor(
                vs2[:], zsq[:], float(KV * EPS), vs[:],
                op0=mybir.AluOpType.mult, op1=mybir.AluOpType.add,
            )
            sd = sm.tile([P, NQ], F32, tag="sd")
            nc.scalar.activation(
                sd[:], vs2[:], AF.Sqrt, bias=0.0, scale=1.0 / KV
            )
            rstd = sm.tile([P, NQ], F32, tag="rstd")
            nc.vector.reciprocal(rstd[:], sd[:])
            cur["cen"], cur["rstd"] = cen, rstd

        def ln2_stage(t):
            # scale/shift on Pool (dep-free at period start) + output DMA
            cur = st.pop(t)
            cen, rstd = cur["cen"], cur["rstd"]
            o_t = outp.tile([P, NQ, KV], F32, tag="o")
            nc.gpsimd.tensor_mul(
                o_t[:], cen[:], rstd[:].unsqueeze(2).broadcast_to((P, NQ, KV))
            )
            nc.gpsimd.tensor_mul(
                o_t[:], o_t[:], g_t.unsqueeze(1).broadcast_to((P, NQ, KV))
            )
            nc.gpsimd.tensor_add(
                o_t[:], o_t[:], b_t.unsqueeze(1).broadcast_to((P, NQ, KV))
            )
            nc.sync.dma_start(
                out=out_d[t * P : (t + 1) * P, :],
                in_=o_t[:].rearrange("p a b -> p (a b)"),
            )

        # pipeline: proj(t) | scores(t-1) | exp+attnv(t-2) | ln1(t-3) | ln2(t-4)
        # emission order per iteration tuned so each engine's in-order queue
        # sees ready work first (exp inputs and ln2 inputs are a period old).
        for t in range(ntiles + 4):
            if 2 <= t < ntiles + 2:
                exp_stage(t - 2)
            if t < ntiles:
                proj_stage(t)
            if 1 <= t < ntiles + 1:
                scores_stage(t - 1)
            if 2 <= t < ntiles + 2:
                attnv_stage(t - 2)
            if 3 <= t < ntiles + 3:
                ln1_stage(t - 3)
            if t >= 4:
                ln2_stage(t - 4)

    _split_multi_waits(nc)
    return nc


def _split_multi_waits(nc):
    """Walrus allows only one sync-wait slot on most instruction encodings.
    Hoist excess waits into NoOps inserted just before the offender (same
    engine, same block => same ordering semantics)."""
    for f in nc.m.functions:
        for b in f.blocks:
            i = 0
            while i < len(b.instructions):
                inst = b.instructions[i]
                si = getattr(inst, "sync_info", None)
                if si is not None and si.on_wait and len(si.on_wait) > 1:
                    extra = si.on_wait[:-1]
                    si.on_wait = si.on_wait[-1:]
                    for w in extra:
                        nop = mybir.InstNoOp(
                            name=nc.get_next_instruction_name(),
                            engine=inst.engine,
                            ins=[],
                            outs=[],
                            sync_info=mybir.SyncInfo(on_wait=[w], on_update=[]),
                        )
                        nc.register_instruction(nop)
                        b.instructions.insert(i, nop)
                        i += 1
                i += 1
    return nc


_NC_CACHE = {}


def _get_program(b_core):
    if b_core not in _NC_CACHE:
        _NC_CACHE[b_core] = build_program(b_core)
    return _NC_CACHE[b_core]


def _host_consts(mask, Wq, bq, Wk, bk, Wv, bv):
    """Build pmat [128, PMAT_COLS]: identity, block-diag projection mats
    (transposed-layout), bias columns. The v blocks/biases are pre-scaled by
    expm[token] = exp(-1e9*mask[token]) so attn@v needs no separate mask
    multiply."""
    gidx = np.empty(NQ, dtype=np.int64)
    for g, (s, e, n) in enumerate(GROUPS):
        gidx[s // KV : e // KV] = g

    expm = np.exp(np.float32(-1e9) * np.asarray(mask, np.float32)).astype(np.float32)

    def mk_blockdiag(W, tok_lo, tok_hi, scale=None):
        n = (tok_hi - tok_lo) * KV
        M = np.zeros((n, n), dtype=np.float32)
        for i in range(tok_lo, tok_hi):
            blk = W[gidx[i]]  # [e, d]
            r = (i - tok_lo) * KV
            s = 1.0 if scale is None else scale[i]
            # lhsT[(n,d'), (i,e)] = W[g(i)][e, d']  -> block at [r:r+9, r:r+9] = W.T
            M[r : r + KV, r : r + KV] = blk.T * s
        return M

    def mk_bias(b_, tok_lo, tok_hi, scale=None):
        parts = []
        for i in range(tok_lo, tok_hi):
            s = 1.0 if scale is None else scale[i]
            parts.append(b_[gidx[i]] * s)
        return np.concatenate(parts).astype(np.float32)

    pmat = np.zeros((P, PMAT_COLS), dtype=np.float32)
    pmat[:, 0:128] = np.eye(P, dtype=np.float32)
    o = 128
    for W, sc in ((Wq, None), (Wk, None), (Wv, expm)):
        pmat[0:NA, o : o + NA] = mk_blockdiag(np.asarray(W, np.float32), 0, 14, sc)
        o += NA
    for W, sc in ((Wq, None), (Wk, None), (Wv, expm)):
        pmat[0:NB, o : o + NB] = mk_blockdiag(np.asarray(W, np.float32), 14, 25, sc)
        o += NB
    for b_, sc in ((bq, None), (bk, None), (bv, expm)):
        pmat[0:NA, o] = mk_bias(np.asarray(b_, np.float32), 0, 14, sc); o += 1
        pmat[0:NB, o] = mk_bias(np.asarray(b_, np.float32), 14, 25, sc); o += 1
    assert o == PMAT_COLS
    return pmat, expm


def kernel(x, mask, Wq, bq, Wk, bk, Wv, bv, gamma, beta):
    x = np.ascontiguousarray(np.asarray(x, dtype=np.float32))
    B = x.shape[0]
    b_core = B // N_CORES
    pmat, expm = _host_consts(mask, Wq, bq, Wk, bk, Wv, bv)
    cst = np.concatenate([
        expm.reshape(-1),
        -np.asarray(gamma, dtype=np.float32).reshape(-1),
        np.asarray(beta, dtype=np.float32).reshape(-1),
    ]).astype(np.float32)
    assert cst.shape[0] == CST_LEN

    nc = _get_program(b_core)
    shards = x.reshape(N_CORES, b_core, D)
    in_maps = []
    for c in range(N_CORES):
        in_maps.append({
            "x": np.ascontiguousarray(shards[c]),
            "cst": cst,
            "pmat": pmat,
        })
    res = run_bass_kernel_spmd(nc, in_maps, core_ids=list(range(N_CORES)))
    outs = [res.results[c]["out"] for c in range(N_CORES)]
    full = np.concatenate(outs, axis=0).reshape(B, NQ, KV)
    return full.astype(np.float32)


# revision 35
# speedup vs baseline: 2.0948x; 1.1969x over previous
"""Trainium2 Bass kernel for nn_AttentionSubModule: batched tiny attention.

Per item (131072 total): x row of 225 = 25 tokens x 9 dims, 4 token groups
each with own 9x9 Wq/Wk/Wv + bias; scores = qk^T/3 (+mask*-1e9), softmax,
out = attn@v + residual, LayerNorm over the 9-dim axis.

Mapping: pure data parallel over 8 cores (16384 items each), 128 items per
SBUF tile (items on partitions), 3-stage software pipeline per tile:

  proj(t+1)  PE transposes + block-diag projection matmuls, ACT evacs
  main(t)    scores products + d-tree, exp, attn@v products + j-tree.
             The big products are SPLIT by query token between the vector
             (DVE) and gpsimd (Pool) engines to balance occupancy.
  back(t-1)  scaled residual, LayerNorm, output DMA (Pool + DVE + ACT).

Tricks:
 - softmax division folded away via LayerNorm scale invariance:
   LN(attn@v/Z + x) == LN(attn_unnorm@v + Z*x).
 - mask weights exp(-1e9*mask) folded host-side into the v projection
   blocks; Z comes out of the attn@v tree via an extra expm row in vE.
 - mean-subtraction sign absorbed into a host-negated gamma so the
   centering step is a single scalar_tensor_tensor.
"""

import numpy as np
from contextlib import ExitStack

import concourse.bass as bass
import concourse.tile as tile
from concourse import mybir
from concourse.bass_utils import run_bass_kernel_spmd

KV = 9
NQ = 25
D = NQ * KV  # 225
GROUPS = [(0, 27, 3), (27, 117, 10), (117, 207, 10), (207, 225, 2)]
N_CORES = 8
P = 128
EPS = 1e-5
F32 = mybir.dt.float32
F16 = mybir.dt.float16

NA = 14 * KV   # chunk A: tokens 0..13 -> 126 rows
NB = 11 * KV   # chunk B: tokens 14..24 -> 99 rows

# DVE/Pool split points: query tokens [0,CUT) of the scores / attn@v
# products go to DVE, [CUT,25) to Pool (gpsimd).
CUT2 = 20
CUT3 = 20

# pmat (per-partition consts) column layout:
#   [0:128)    identity 128x128
#   [128:254)  MqA  (126x126)    [254:380) MkA   [380:506) MvA
#   [506:605)  MqB  (99x99)      [605:704) MkB   [704:803) MvB
#   803 bqA | 804 bqB | 805 bkA | 806 bkB | 807 bvA | 808 bvB
PMAT_COLS = 128 + 3 * NA + 3 * NB + 6  # 809

# cst (broadcast consts): [expm 25 | -gamma 9 | beta 9]
CST_LEN = NQ + KV + KV


def _bcast_ap(handle, n_part):
    ap = handle[:]
    return bass.AP(tensor=ap.tensor, offset=ap.offset, ap=[[0, n_part]] + list(ap.ap))


def build_program(b_core):
    assert b_core % P == 0
    ntiles = b_core // P
    nc = bass.Bass("TRN2", target_bir_lowering=False)

    x_d = nc.dram_tensor("x", [b_core, D], F32, kind="ExternalInput")
    cst_d = nc.dram_tensor("cst", [CST_LEN], F32, kind="ExternalInput")
    pmat_d = nc.dram_tensor("pmat", [P, PMAT_COLS], F32, kind="ExternalInput")
    out_d = nc.dram_tensor("out", [b_core, D], F32, kind="ExternalOutput")

    with tile.TileContext(nc) as tc, ExitStack() as ctx:
        consts = ctx.enter_context(tc.tile_pool(name="consts", bufs=1))
        xin = ctx.enter_context(tc.tile_pool(name="xin", bufs=6))
        tlay = ctx.enter_context(tc.tile_pool(name="tlay", bufs=2))
        proj = ctx.enter_context(tc.tile_pool(name="proj", bufs=4))
        big = ctx.enter_context(tc.tile_pool(name="big", bufs=3))
        sm = ctx.enter_context(tc.tile_pool(name="sm", bufs=3))
        outp = ctx.enter_context(tc.tile_pool(name="outp", bufs=3))
        psum = ctx.enter_context(tc.tile_pool(name="psum", bufs=2, space="PSUM"))
        pacc = ctx.enter_context(tc.tile_pool(name="pacc", bufs=2, space="PSUM"))

        # ---- constants ----
        cst_t = consts.tile([P, CST_LEN], F32)
        nc.gpsimd.dma_start(out=cst_t, in_=_bcast_ap(cst_d, P))
        m_t = cst_t[:, 0:NQ]          # expm = exp(-1e9*mask), host-computed
        g_t = cst_t[:, NQ : NQ + KV]  # -gamma (sign absorbs mu-u centering)
        b_t = cst_t[:, NQ + KV : NQ + 2 * KV]

        pm_t = consts.tile([P, PMAT_COLS], F32)
        nc.sync.dma_start(out=pm_t, in_=pmat_d[:, :])
        ident = pm_t[:, 0:128]
        mA = {}
        mB = {}
        o = 128
        for nm in ("q", "k", "v"):
            mA[nm] = pm_t[0:NA, o : o + NA]; o += NA
        for nm in ("q", "k", "v"):
            mB[nm] = pm_t[0:NB, o : o + NB]; o += NB
        biasA = {}
        biasB = {}
        for nm in ("q", "k", "v"):
            biasA[nm] = pm_t[0:NA, o : o + 1]; o += 1
            biasB[nm] = pm_t[0:NB, o : o + 1]; o += 1
        assert o == PMAT_COLS

        # fp16 identity: stationary for PE accumulate-copy reduction matmuls
        identF16 = consts.tile([P, P], F16)
        nc.vector.tensor_copy(identF16[:], ident)

        # constant exp-shift: exp(s - 8) keeps fp16 attention weights and
        # partial sums in range (max score ~16.4); softmax is shift-invariant
        # and the LN scale-invariance absorbs the global factor exactly.
        shift_t = consts.tile([P, 1], F32)
        nc.vector.memset(shift_t[:], -8.0)
        # wait-absorbers: sync engines on the const DMAs via tiny copies so
        # wide TensorTensor encodings never need more than one sync-wait.
        absorb_t = consts.tile([P, 4], F32)
        nc.vector.tensor_copy(absorb_t[:], cst_t[:, 0:4])
        absorb2_t = consts.tile([P, 4], F32)
        nc.vector.tensor_copy(absorb2_t[:], pm_t[:, 0:4])
        absorb3_t = consts.tile([P, 4], F32)
        nc.gpsimd.tensor_copy(absorb3_t[:], cst_t[:, 0:4])

        inv_sqrt_kv = float(1.0 / np.sqrt(KV))
        AF = mybir.ActivationFunctionType
        st = {}  # per-tile live handles

        def proj_stage(t):
            h = t % 2
            if h == 0:
                x2 = xin.tile([P, 2, D], F32, tag="x")
            else:
                x2 = st[t - 1]["x2"]
            nc.sync.dma_start(
                out=x2[:, h : h + 1, :].rearrange("p a b -> p (a b)"),
                in_=x_d[t * P : (t + 1) * P, :])

            xt = x2[:, h, :]
            xTp = psum.tile([P, 2 * P], F32, tag="ps")
            nc.tensor.transpose(xTp[0:NA, 0:P], xt[:, 0:NA], ident)
            nc.tensor.transpose(xTp[0:NB, P : 2 * P], xt[:, NA:D], ident)
            xTm = tlay.tile([P, 2 * P], F32, tag="xTm")
            nc.scalar.copy(xTm[0:NA, :], xTp[0:NA, :])
            xT1 = xTm[:, 0:P]
            xT2 = xTm[:, P : 2 * P]

            cur = {"x2": x2}
            for nm in ("q", "k", "v"):
                pA = psum.tile([P, P], F32, tag="ps")
                pB = psum.tile([P, P], F32, tag="ps")
                nc.tensor.matmul(pA[0:NA, :], mA[nm], xT1[0:NA, 0:P], start=True, stop=True)
                nc.tensor.matmul(pB[0:NB, :], mB[nm], xT2[0:NB, 0:P], start=True, stop=True)
                sT1 = tlay.tile([P, P], F32, tag=f"s{nm}1")
                sT2 = tlay.tile([P, P], F32, tag=f"s{nm}2")
                nc.scalar.activation(sT1[0:NA, :], pA[0:NA, :], AF.Identity,
                                     bias=biasA[nm], scale=1.0)
                nc.scalar.activation(sT2[0:NB, :], pB[0:NB, :], AF.Identity,
                                     bias=biasB[nm], scale=1.0)
                nAB = psum.tile([P, D], F32, tag="ps")
                nc.tensor.transpose(nAB[:, 0:NA], sT1[0:NA, :], ident[0:NA, 0:NA])
                nc.tensor.transpose(nAB[:, NA:D], sT2[0:NB, :], ident[0:NB, 0:NB])
                if nm in ("q", "k"):
                    dst = proj.tile([P, NQ, KV], F16, tag=nm)
                    nc.scalar.copy(dst[:].rearrange("p a b -> p (a b)"), nAB[:])
                    cur[nm] = dst
                else:
                    # v straight to e-major layout [P, 10, 25]:
                    # rows 0..8 = premasked v, row 9 = expm weights (-> Z)
                    vE = proj.tile([P, KV + 1, NQ], F16, tag="ve")
                    nc.scalar.copy(
                        vE[:, 0:KV, :].transpose([0, 2, 1]),
                        nAB[:].rearrange("p (j e) -> p j e", j=NQ),
                    )
                    nc.scalar.copy(vE[:, KV : KV + 1, :], m_t.unsqueeze(1))
                    cur["ve"] = vE
            st[t] = cur

        NSA = 512                  # scores column split at the PSUM bank edge
        NSB = NQ * NQ - NSA        # 113
        NR = NQ * (KV + 1)         # 250 attn@v reduce groups
        SC_PRE = 4                 # d-pairs folded on DVE/Pool before PE
        AV_PRE = 12                # j-pairs folded on DVE/Pool before PE

        def scores_stage(t):
            # pr2 products + pair-add, split DVE/Pool by query token (each
            # engine's chain is local); PE accumulates 5 slices -> PSUM f32.
            cur = st[t]
            q_t, k_t = cur["q"], cur["k"]
            pr2 = big.tile([P, NQ, NQ, KV], F16, tag="pr2")
            qb = q_t[:].unsqueeze(2).broadcast_to((P, NQ, NQ, KV))
            kb = k_t[:].unsqueeze(1).broadcast_to((P, NQ, NQ, KV))
            nc.vector.tensor_mul(pr2[:, 0:CUT2], qb[:, 0:CUT2], kb[:, 0:CUT2])
            nc.gpsimd.tensor_mul(pr2[:, CUT2:NQ], qb[:, CUT2:NQ], kb[:, CUT2:NQ])
            if SC_PRE:
                t1 = sm.tile([P, NQ, NQ, SC_PRE], F16, tag="sct1")
                nc.vector.tensor_add(t1[:, 0:CUT2], pr2[:, 0:CUT2, :, 0:SC_PRE],
                                     pr2[:, 0:CUT2, :, SC_PRE : 2 * SC_PRE])
                nc.gpsimd.tensor_add(t1[:, CUT2:NQ], pr2[:, CUT2:NQ, :, 0:SC_PRE],
                                     pr2[:, CUT2:NQ, :, SC_PRE : 2 * SC_PRE])
                t1f = t1[:].rearrange("p a b c -> p (a b) c")
            p4 = pr2[:].rearrange("p a b d -> p (a b) d")
            scp = pacc.tile([P, NQ * NQ], F32, tag="scp")
            for lo, n in ((0, NSA), (NSA, NSB)):
                srcs = ([t1f[:, lo : lo + n, c : c + 1] for c in range(SC_PRE)]
                        if SC_PRE else [])
                srcs += [p4[:, lo : lo + n, c : c + 1]
                         for c in range(2 * SC_PRE, KV)]
                for ci, s in enumerate(srcs):
                    nc.tensor.matmul(scp[:, lo : lo + n], identF16[:], s,
                                     start=(ci == 0), stop=(ci == len(srcs) - 1))
            cur["scp"] = scp

        def exp_stage(t):
            # exp from PSUM scores (inputs one period old -> ACT queue head)
            cur = st[t]
            scp = cur.pop("scp")
            ex = sm.tile([P, NQ, NQ], F16, tag="ex")
            nc.scalar.activation(ex[:].rearrange("p a b -> p (a b)"), scp[:],
                                 AF.Exp, bias=shift_t[:], scale=inv_sqrt_kv)
            cur["ex"] = ex

        def attnv_stage(t):
            # pr3 products + pair-add (DVE/Pool local chains),
            # PE accumulates 13 slices -> u10 PSUM (evac next period).
            # u10[p,i,e] = sum_j ex[p,i,j] vE[p,e,j]; e==9 row gives Z.
            cur = st[t]
            vE = cur["ve"]
            ex = cur.pop("ex")
            pr3 = big.tile([P, NQ, KV + 1, NQ], F16, tag="pr3")
            exb = ex[:].unsqueeze(2).broadcast_to((P, NQ, KV + 1, NQ))
            veb = vE[:].unsqueeze(1).broadcast_to((P, NQ, KV + 1, NQ))
            nc.vector.tensor_mul(pr3[:, 0:CUT3], exb[:, 0:CUT3], veb[:, 0:CUT3])
            nc.gpsimd.tensor_mul(pr3[:, CUT3:NQ], exb[:, CUT3:NQ], veb[:, CUT3:NQ])
            if AV_PRE:
                r1 = sm.tile([P, NQ, KV + 1, AV_PRE], F16, tag="avr1")
                nc.vector.tensor_add(r1[:, 0:CUT3], pr3[:, 0:CUT3, :, 0:AV_PRE],
                                     pr3[:, 0:CUT3, :, AV_PRE : 2 * AV_PRE])
                nc.gpsimd.tensor_add(r1[:, CUT3:NQ], pr3[:, CUT3:NQ, :, 0:AV_PRE],
                                     pr3[:, CUT3:NQ, :, AV_PRE : 2 * AV_PRE])
                r1f = r1[:].rearrange("p a b c -> p (a b) c")
            q3 = pr3[:].rearrange("p a b d -> p (a b) d")
            u10p = pacc.tile([P, NR + NQ], F32, tag="u10p")
            srcs = [r1f[:, :, c : c + 1] for c in range(AV_PRE)] if AV_PRE else []
            srcs += [q3[:, :, c : c + 1] for c in range(2 * AV_PRE, NQ)]
            for ci, s in enumerate(srcs):
                nc.tensor.matmul(u10p[:, 0:NR], identF16[:], s,
                                 start=(ci == 0), stop=(ci == len(srcs) - 1))
            cur["u10p"] = u10p

        def evac_stage(t):
            # per-tile u10 PSUM evac into half of a paired SBUF tile
            cur = st[t]
            h = t % 2
            if h == 0:
                u10x2 = outp.tile([P, 2, NQ, KV + 1], F32, tag="u10")
                cur["u10x2"] = u10x2
            else:
                u10x2 = st[t - 1]["u10x2"]
                cur["u10x2"] = u10x2
            u10p = cur.pop("u10p")
            nc.scalar.copy(
                u10x2[:, h : h + 1].rearrange("p h a b -> p (h a b)"),
                u10p[:, 0:NR])

        def ln1_pair(a):
            # residual + LayerNorm stats for tiles (a, a+1) in one op each
            cur = st[a]
            u10x2 = cur["u10x2"]
            xv2 = cur["x2"][:].rearrange("p h (i d) -> p h i d", i=NQ)
            uv2 = u10x2[:, :, :, 0:KV]
            zv2 = u10x2[:, :, :, KV : KV + 1]

            # u += Z * x   (residual, scaled by Z; LN is scale-invariant)
            zx2 = outp.tile([P, 2, NQ, KV], F32, tag="zx")
            nc.vector.tensor_mul(zx2[:], xv2, zv2.broadcast_to((P, 2, NQ, KV)))
            nc.vector.tensor_add(uv2, uv2, zx2[:])
            zsq2 = sm.tile([P, 2, NQ], F32, tag="zsq")
            nc.scalar.square(zsq2[:], zv2.rearrange("p h a b -> p h (a b)"))

            s_t2 = sm.tile([P, 2, NQ], F32, tag="lnsum")
            nc.vector.tensor_reduce(
                s_t2[:], uv2, axis=mybir.AxisListType.X, op=mybir.AluOpType.add
            )
            # cen' = mu - u  (negated centering; sign folded into -gamma)
            cen2 = outp.tile([P, 2, NQ, KV], F32, tag="cen")
            nc.vector.scalar_tensor_tensor(
                cen2[:], s_t2[:].unsqueeze(3).broadcast_to((P, 2, NQ, KV)),
                float(1.0 / KV), uv2,
                op0=mybir.AluOpType.mult, op1=mybir.AluOpType.subtract,
            )
            sq2 = outp.tile([P, 2, NQ, KV], F32, tag="sq")
            nc.scalar.square(
                sq2[:].rearrange("p h a b -> p (h a b)"),
                cen2[:].rearrange("p h a b -> p (h a b)")
            )
            vs2_ = sm.tile([P, 2, NQ], F32, tag="vs")
            nc.vector.tensor_reduce(
                vs2_[:], sq2[:], axis=mybir.AxisListType.X, op=mybir.AluOpType.add
            )
            # scale-correct eps: u = Z*out_ref, so var_ref = var_u / Z^2;
            # rstd_ref/Z = 1/sqrt(var_u + Z^2*eps)
            vs2b = sm.tile([P, 2, NQ], F32, tag="vs2")
            nc.vector.scalar_tensor_tensor(
                vs2b[:], zsq2[:], float(KV * EPS), vs2_[:],
                op0=mybir.AluOpType.mult, op1=mybir.AluOpType.add,
            )
            sd2 = sm.tile([P, 2, NQ], F32, tag="sd")
            nc.scalar.activation(
                sd2[:], vs2b[:], AF.Sqrt, bias=0.0, scale=1.0 / KV
            )
            rstd2 = sm.tile([P, 2, NQ], F32, tag="rstd")
            nc.vector.reciprocal(rstd2[:], sd2[:])
            cur["cen2"], cur["rstd2"] = cen2, rstd2

        def ln2_pair(a):
            # scale/shift on Pool for both tiles + one paired output DMA
            cur = st.pop(a)
            st.pop(a + 1)
            cen2, rstd2 = cur["cen2"], cur["rstd2"]
            o2 = outp.tile([P, 2, NQ, KV], F32, tag="o")
            nc.gpsimd.tensor_mul(
                o2[:], cen2[:], rstd2[:].unsqueeze(3).broadcast_to((P, 2, NQ, KV))
            )
            nc.gpsimd.tensor_mul(
                o2[:], o2[:],
                g_t.unsqueeze(1).unsqueeze(1).broadcast_to((P, 2, NQ, KV))
            )
            nc.gpsimd.tensor_add(
                o2[:], o2[:],
                b_t.unsqueeze(1).unsqueeze(1).broadcast_to((P, 2, NQ, KV))
            )
            nc.sync.dma_start(
                out=out_d[a * P : (a + 2) * P, :].rearrange(
                    "(h p) c -> p h c", h=2),
                in_=o2[:].rearrange("p h a b -> p h (a b)"),
            )

        # pipeline: proj(t) | scores(t-1) | exp+attnv(t-2) | evac(t-3) |
        # ln1(pair t-5,t-4) | ln2(pair, +1). LN stages process tile PAIRS to
        # halve per-instruction fixed costs. Emission order per iteration is
        # tuned so each engine's in-order queue sees ready work first.
        assert ntiles % 2 == 0
        for t in range(ntiles + 7):
            if 2 <= t < ntiles + 2:
                exp_stage(t - 2)
            if t < ntiles:
                proj_stage(t)
            if 1 <= t < ntiles + 1:
                scores_stage(t - 1)
            if 2 <= t < ntiles + 2:
                attnv_stage(t - 2)
            if 3 <= t < ntiles + 3:
                evac_stage(t - 3)
            if 5 <= t and (t - 5) % 2 == 0 and t - 5 < ntiles:
                ln1_pair(t - 5)
            if 7 <= t and (t - 7) % 2 == 0 and t - 7 < ntiles:
                ln2_pair(t - 7)

    _split_multi_waits(nc)
    return nc


def _split_multi_waits(nc):
    """Walrus allows only one sync-wait slot on most instruction encodings.
    Hoist excess waits into NoOps inserted just before the offender (same
    engine, same block => same ordering semantics)."""
    for f in nc.m.functions:
        for b in f.blocks:
            i = 0
            while i < len(b.instructions):
                inst = b.instructions[i]
                si = getattr(inst, "sync_info", None)
                if si is not None and si.on_wait and len(si.on_wait) > 1:
                    extra = si.on_wait[:-1]
                    si.on_wait = si.on_wait[-1:]
                    for w in extra:
                        nop = mybir.InstNoOp(
                            name=nc.get_next_instruction_name(),
                            engine=inst.engine,
                            ins=[],
                            outs=[],
                            sync_info=mybir.SyncInfo(on_wait=[w], on_update=[]),
                        )
                        nc.register_instruction(nop)
                        b.instructions.insert(i, nop)
                        i += 1
                i += 1
    return nc


_NC_CACHE = {}


def _get_program(b_core):
    if b_core not in _NC_CACHE:
        _NC_CACHE[b_core] = build_program(b_core)
    return _NC_CACHE[b_core]


def _host_consts(mask, Wq, bq, Wk, bk, Wv, bv):
    """Build pmat [128, PMAT_COLS]: identity, block-diag projection mats
    (transposed-layout), bias columns. The v blocks/biases are pre-scaled by
    expm[token] = exp(-1e9*mask[token]) so attn@v needs no separate mask
    multiply."""
    gidx = np.empty(NQ, dtype=np.int64)
    for g, (s, e, n) in enumerate(GROUPS):
        gidx[s // KV : e // KV] = g

    expm = np.exp(np.float32(-1e9) * np.asarray(mask, np.float32)).astype(np.float32)

    def mk_blockdiag(W, tok_lo, tok_hi, scale=None):
        n = (tok_hi - tok_lo) * KV
        M = np.zeros((n, n), dtype=np.float32)
        for i in range(tok_lo, tok_hi):
            blk = W[gidx[i]]  # [e, d]
            r = (i - tok_lo) * KV
            s = 1.0 if scale is None else scale[i]
            # lhsT[(n,d'), (i,e)] = W[g(i)][e, d']  -> block at [r:r+9, r:r+9] = W.T
            M[r : r + KV, r : r + KV] = blk.T * s
        return M

    def mk_bias(b_, tok_lo, tok_hi, scale=None):
        parts = []
        for i in range(tok_lo, tok_hi):
            s = 1.0 if scale is None else scale[i]
            parts.append(b_[gidx[i]] * s)
        return np.concatenate(parts).astype(np.float32)

    pmat = np.zeros((P, PMAT_COLS), dtype=np.float32)
    pmat[:, 0:128] = np.eye(P, dtype=np.float32)
    o = 128
    for W, sc in ((Wq, None), (Wk, None), (Wv, expm)):
        pmat[0:NA, o : o + NA] = mk_blockdiag(np.asarray(W, np.float32), 0, 14, sc)
        o += NA
    for W, sc in ((Wq, None), (Wk, None), (Wv, expm)):
        pmat[0:NB, o : o + NB] = mk_blockdiag(np.asarray(W, np.float32), 14, 25, sc)
        o += NB
    for b_, sc in ((bq, None), (bk, None), (bv, expm)):
        pmat[0:NA, o] = mk_bias(np.asarray(b_, np.float32), 0, 14, sc); o += 1
        pmat[0:NB, o] = mk_bias(np.asarray(b_, np.float32), 14, 25, sc); o += 1
    assert o == PMAT_COLS
    return pmat, expm


def kernel(x, mask, Wq, bq, Wk, bk, Wv, bv, gamma, beta):
    x = np.ascontiguousarray(np.asarray(x, dtype=np.float32))
    B = x.shape[0]
    b_core = B // N_CORES
    pmat, expm = _host_consts(mask, Wq, bq, Wk, bk, Wv, bv)
    cst = np.concatenate([
        expm.reshape(-1),
        -np.asarray(gamma, dtype=np.float32).reshape(-1),
        np.asarray(beta, dtype=np.float32).reshape(-1),
    ]).astype(np.float32)
    assert cst.shape[0] == CST_LEN

    nc = _get_program(b_core)
    shards = x.reshape(N_CORES, b_core, D)
    in_maps = []
    for c in range(N_CORES):
        in_maps.append({
            "x": np.ascontiguousarray(shards[c]),
            "cst": cst,
            "pmat": pmat,
        })
    res = run_bass_kernel_spmd(nc, in_maps, core_ids=list(range(N_CORES)))
    outs = [res.results[c]["out"] for c in range(N_CORES)]
    full = np.concatenate(outs, axis=0).reshape(B, NQ, KV)
    return full.astype(np.float32)
